# revision 6
# baseline (speedup 1.0000x reference)
"""Trainium2 Bass kernel for nn_MultiHeadAttention_88192858456426.

Reference computation (per batch b, C=512 channels, N=2048 tokens):
    qp = wq @ q + bq          # [C, N]
    kp = wk @ k + bk          # [C, N]
    vp = wv @ v + bv          # [C, N]
    S[m, n]  = sum_c kp[c, m] * qp[c, n]        # QK^T (transposed view)
    out[c,n] = sum_m vp[c, m] * S[m, n] + q[c, n]

Sharding: data-parallel over batch B=8 across the 8 NeuronCores (one batch
per core, no collectives).

Per-core dataflow (all matmuls as out[M,Nf] = lhsT[K,M].T @ rhs[K,Nf]):
  - kp[c, m]:  lhsT = wkT[i, c] chunk, rhs = k[i, m] chunk  (+bk via ACT bias)
  - vpt[m, c]: lhsT = v[i, m] chunk,  rhs = wvT[i, c]       (+bv via DVE add
               of a host-broadcast [128, C] tile)
  - per n-block of 512 columns:
      qp[c, n]:  lhsT = wqT chunk, rhs = q[i, n] chunk (+bq via ACT bias)
      S[m, n]:   lhsT = kp[c, m128] slice, rhs = qp[c, nb]
      out[c, n]: lhsT = vpt[m, c128] slice, rhs = S[m, nb], accumulated
                 over all 16 m-chunks in PSUM, then +q residual on DVE.

Matmul operands are bitcast to float32r (fp32 data, reduced-precision PE
mode) which streams at 1 column/cycle vs 4 for plain fp32.
"""

import numpy as np
from contextlib import ExitStack

import concourse.bass as bass
import concourse.mybir as mybir
import concourse.tile as tile
from concourse import bacc
from concourse.bass_utils import run_bass_kernel_spmd

P = 128            # partitions
C = 512            # channels
N = 2048           # tokens
NB = 512           # n-block width (one PSUM bank of fp32)
CK = C // P        # 4 channel chunks
MCH = N // P       # 16 token chunks
NBK = N // NB      # 4 n-blocks
NH = N // 2        # half of tokens (raw k/v staging granularity)

F32 = mybir.dt.float32
F32R = mybir.dt.float32r
ACT_IDENT = mybir.ActivationFunctionType.Identity

N_CORES = 8




def build_nc():
    nc = bacc.Bacc("TRN2", target_bir_lowering=False, debug=False,
                   num_devices=N_CORES)

    q_d = nc.dram_tensor("q", [C, N], F32R, kind="ExternalInput").ap()
    k_d = nc.dram_tensor("k", [C, N], F32R, kind="ExternalInput").ap()
    v_d = nc.dram_tensor("v", [C, N], F32R, kind="ExternalInput").ap()
    wqT_d = nc.dram_tensor("wqT", [C, C], F32R, kind="ExternalInput").ap()
    wkT_d = nc.dram_tensor("wkT", [C, C], F32R, kind="ExternalInput").ap()
    wvT_d = nc.dram_tensor("wvT", [C, C], F32R, kind="ExternalInput").ap()
    bqt_d = nc.dram_tensor("bqt", [P, CK], F32, kind="ExternalInput").ap()
    bkt_d = nc.dram_tensor("bkt", [P, CK], F32, kind="ExternalInput").ap()
    bvb_d = nc.dram_tensor("bvb", [P, C], F32, kind="ExternalInput").ap()
    o_d = nc.dram_tensor("o", [C, N], F32, kind="ExternalOutput").ap()

    with ExitStack() as ctx:
        tc = ctx.enter_context(tile.TileContext(nc))
        consts = ctx.enter_context(tc.tile_pool(name="consts", bufs=1))
        wpool = ctx.enter_context(tc.tile_pool(name="wpool", bufs=1))
        kvraw = ctx.enter_context(tc.tile_pool(name="kvraw", bufs=2))
        persist = ctx.enter_context(tc.tile_pool(name="persist", bufs=1))
        qpool = ctx.enter_context(tc.tile_pool(name="qpool", bufs=2))
        qppool = ctx.enter_context(tc.tile_pool(name="qppool", bufs=2))
        spool = ctx.enter_context(tc.tile_pool(name="spool", bufs=3))
        opool = ctx.enter_context(tc.tile_pool(name="opool", bufs=2))
        ps_a = ctx.enter_context(tc.tile_pool(name="ps_a", bufs=2, space="PSUM"))
        ps_s = ctx.enter_context(tc.tile_pool(name="ps_s", bufs=2, space="PSUM"))
        ps_r = ctx.enter_context(tc.tile_pool(name="ps_r", bufs=4, space="PSUM"))

        # ---- constants ----
        bqt = consts.tile([P, CK], F32, tag="bqt", name="bqt")
        nc.sync.dma_start(bqt[:], bqt_d[:])
        bkt = consts.tile([P, CK], F32, tag="bkt", name="bkt")
        nc.sync.dma_start(bkt[:], bkt_d[:])
        bvb = consts.tile([P, C], F32, tag="bvb", name="bvb")
        nc.sync.dma_start(bvb[:], bvb_d[:])

        wq_sb, wk_sb, wv_sb = [], [], []
        for i in range(CK):
            for lst, name, src in ((wq_sb, "wq", wqT_d), (wk_sb, "wk", wkT_d),
                                   (wv_sb, "wv", wvT_d)):
                t = wpool.tile([P, C], F32R, tag=f"{name}{i}", name=f"{name}{i}")
                nc.sync.dma_start(t[:], src[i * P:(i + 1) * P, :])
                lst.append(t)

        # ---- phase A: kp[c, m] = wk @ k + bk, kept in SBUF ----
        kp_sb = [persist.tile([P, N], F32R, tag=f"kp{c}", name=f"kp{c}") for c in range(CK)]
        for h in range(2):
            kh = []
            for i in range(CK):
                t = kvraw.tile([P, NH], F32R, tag=f"kv{i}", name=f"kv{i}")
                nc.sync.dma_start(t[:], k_d[i * P:(i + 1) * P,
                                            h * NH:(h + 1) * NH])
                kh.append(t)
            for c in range(CK):
                for s in range(NH // NB):
                    ps = ps_a.tile([P, NB], F32, tag="ps_a", name="ps_a")
                    for i in range(CK):
                        nc.tensor.matmul(
                            ps[:],
                            (wk_sb[i][:, c * P:(c + 1) * P]),
                            (kh[i][:, s * NB:(s + 1) * NB]),
                            start=(i == 0), stop=(i == CK - 1))
                    nc.scalar.activation(
                        kp_sb[c][:, h * NH + s * NB: h * NH + (s + 1) * NB],
                        ps[:], ACT_IDENT, bias=bkt[:, c:c + 1])

        # ---- phase B: vpt[m, c] = (wv @ v + bv)^T, kept in SBUF ----
        vpt_sb = [persist.tile([P, C], F32R, tag=f"vpt{m}", name=f"vpt{m}") for m in range(MCH)]
        for h in range(2):
            vh = []
            for i in range(CK):
                t = kvraw.tile([P, NH], F32R, tag=f"kv{i}", name=f"kv{i}")
                nc.sync.dma_start(t[:], v_d[i * P:(i + 1) * P,
                                            h * NH:(h + 1) * NH])
                vh.append(t)
            for ml in range(MCH // 2):
                m = h * (MCH // 2) + ml
                ps = ps_s.tile([P, C], F32, tag="ps_s", name="ps_s")
                for i in range(CK):
                    nc.tensor.matmul(
                        ps[:],
                        (vh[i][:, ml * P:(ml + 1) * P]),
                        (wv_sb[i][:]),
                        start=(i == 0), stop=(i == CK - 1))
                nc.vector.tensor_add(vpt_sb[m][:], ps[:], bvb[:])

        # ---- phase C: per n-block: qp, S, out ----
        for b in range(NBK):
            qt = []
            for i in range(CK):
                t = qpool.tile([P, NB], F32R, tag=f"qt{i}", name=f"qt{i}")
                nc.sync.dma_start(t[:], q_d[i * P:(i + 1) * P,
                                            b * NB:(b + 1) * NB])
                qt.append(t)

            qp_sb = []
            for c in range(CK):
                ps = ps_a.tile([P, NB], F32, tag="ps_a", name="ps_a")
                for i in range(CK):
                    nc.tensor.matmul(
                        ps[:],
                        (wq_sb[i][:, c * P:(c + 1) * P]),
                        (qt[i][:]),
                        start=(i == 0), stop=(i == CK - 1))
                qp = qppool.tile([P, NB], F32R, tag=f"qp{c}", name=f"qp{c}")
                nc.scalar.activation(qp[:], ps[:], ACT_IDENT,
                                     bias=bqt[:, c:c + 1])
                qp_sb.append(qp)

            r_ps = [ps_r.tile([P, NB], F32, tag="ps_r", name="ps_r") for _ in range(CK)]

            def emit_s(m):
                ps = ps_s.tile([P, NB], F32, tag="ps_s", name="ps_s")
                for c in range(CK):
                    nc.tensor.matmul(
                        ps[:],
                        (kp_sb[c][:, m * P:(m + 1) * P]),
                        (qp_sb[c][:]),
                        start=(c == 0), stop=(c == CK - 1))
                return ps

            s_ps_prev = emit_s(0)
            for m in range(MCH):
                s_ps_next = emit_s(m + 1) if m + 1 < MCH else None
                s_sb = spool.tile([P, NB], F32R, tag="s", name="s")
                # alternate PSUM->SBUF copies between ACT and DVE
                if m % 2 == 0:
                    nc.scalar.copy(s_sb[:], s_ps_prev[:])
                else:
                    nc.vector.tensor_copy(s_sb[:], s_ps_prev[:])
                for c in range(CK):
                    nc.tensor.matmul(
                        r_ps[c][:],
                        (vpt_sb[m][:, c * P:(c + 1) * P]),
                        (s_sb[:]),
                        start=(m == 0), stop=(m == MCH - 1))
                s_ps_prev = s_ps_next

            for c in range(CK):
                o_sb = opool.tile([P, NB], F32, tag="o", name="o")
                nc.vector.tensor_add(o_sb[:], r_ps[c][:], qt[c][:].bitcast(F32))
                nc.sync.dma_start(o_d[c * P:(c + 1) * P,
                                      b * NB:(b + 1) * NB], o_sb[:])

    nc.finalize()
    return nc


_CACHE = {}


def _get_nc():
    if "nc" not in _CACHE:
        _CACHE["nc"] = build_nc()
    return _CACHE["nc"]


def _in_maps(q, k, v, wq, bq, wk, bk, wv, bv):
    f32 = lambda x: np.ascontiguousarray(np.asarray(x), dtype=np.float32)
    q, k, v = f32(q), f32(k), f32(v)
    wqT = f32(np.asarray(wq).T)
    wkT = f32(np.asarray(wk).T)
    wvT = f32(np.asarray(wv).T)
    bqt = f32(np.asarray(bq).reshape(CK, P).T)
    bkt = f32(np.asarray(bk).reshape(CK, P).T)
    bvb = f32(np.tile(np.asarray(bv)[None, :], (P, 1)))
    return [
        {"q": q[i], "k": k[i], "v": v[i],
         "wqT": wqT, "wkT": wkT, "wvT": wvT,
         "bqt": bqt, "bkt": bkt, "bvb": bvb}
        for i in range(N_CORES)
    ]


def run(inputs, **spmd_kwargs):
    """Run on hardware; returns (output [B,C,N], BassKernelResults)."""
    nc = _get_nc()
    maps = _in_maps(**inputs)
    res = run_bass_kernel_spmd(nc, maps, list(range(N_CORES)), **spmd_kwargs)
    out = np.stack([res.results[i]["o"] for i in range(N_CORES)], axis=0)
    return out, res


def kernel(q, k, v, wq, bq, wk, bk, wv, bv):
    out, _ = run(dict(q=q, k=k, v=v, wq=wq, bq=bq, wk=wk, bk=bk,
                      wv=wv, bv=bv))
    return out


# revision 29
# speedup vs baseline: 446.3764x; 446.3764x over previous
"""Trainium2 Bass kernel for nn_MultiHeadAttention_88192858456426.

Reference computation (per batch b, C=512 channels, N=2048 tokens):
    qp = wq @ q + bq          # [C, N]
    kp = wk @ k + bk          # [C, N]
    vp = wv @ v + bv          # [C, N]
    S[m, n]  = sum_c kp[c, m] * qp[c, n]        # QK^T (transposed view)
    out[c,n] = sum_m vp[c, m] * S[m, n] + q[c, n]

Sharding: data-parallel over batch B=8 across the 8 NeuronCores (one batch
per core, no collectives).

Per-core dataflow (all matmuls as out[M,Nf] = lhsT[K,M].T @ rhs[K,Nf]):
  - kp[c, m]:  lhsT = wkT[i, c] chunk, rhs = k[i, m] chunk  (+bk via ACT bias)
  - vpt[m, c]: lhsT = v[i, m] chunk,  rhs = wvT[i, c]       (+bv via DVE add
               of a host-broadcast [128, C] tile)
  - per n-block of 512 columns:
      qp[c, n]:  lhsT = wqT chunk, rhs = q[i, n] chunk (+bq via ACT bias)
      S[m, n]:   lhsT = kp[c, m128] slice, rhs = qp[c, nb]
      out[c, n]: lhsT = vpt[m, c128] slice, rhs = S[m, nb], accumulated
                 over all 16 m-chunks in PSUM, then +q residual on DVE.

All matmul operands are float32r (fp32 bits, reduced-precision PE mode,
1 column/cycle vs 4 for plain fp32). The BIR verifier requires the whole
producer chain of a f32r matmul operand to be tagged f32r, so DRAM inputs
and intermediate SBUF tiles carry that dtype; PSUM stays fp32.
"""

import numpy as np
from contextlib import ExitStack

import concourse.bass as bass
import concourse.mybir as mybir
import concourse.tile as tile
from concourse import bacc
from concourse.bass_utils import run_bass_kernel_spmd

P = 128            # partitions
C = 512            # channels
N = 2048           # tokens
NB = 512           # n-block width (one PSUM bank of fp32)
CK = C // P        # 4 channel chunks
MCH = N // P       # 16 token chunks
NBK = N // NB      # 4 n-blocks
NH = N // 2        # half of tokens (raw k/v staging granularity)
_CW = [512, 512, 512, 512]   # phase-C block widths
CBLOCKS = []
_o = 0
for _w in _CW:
    CBLOCKS.append((_o, _w))
    _o += _w
assert _o == N

F32 = mybir.dt.float32
F32R = mybir.dt.float32r
BF16 = mybir.dt.bfloat16
FP16 = mybir.dt.float16
ACT_IDENT = mybir.ActivationFunctionType.Identity

N_CORES = 8


def build_nc(reps=1, mode="f32r"):
    MDT = {"f32r": F32R, "bf16": BF16, "fp16": FP16}[mode]
    QDT = MDT
    nc = bacc.Bacc("TRN2", target_bir_lowering=False, debug=False,
                   num_devices=N_CORES)

    q_d = nc.dram_tensor("q", [C, N], QDT, kind="ExternalInput").ap()
    k_d = nc.dram_tensor("k", [C, N], MDT, kind="ExternalInput").ap()
    v_d = nc.dram_tensor("v", [C, N], MDT, kind="ExternalInput").ap()
    wqT_d = nc.dram_tensor("wqT", [C, C], MDT, kind="ExternalInput").ap()
    wkT_d = nc.dram_tensor("wkT", [C, C], MDT, kind="ExternalInput").ap()
    wvT_d = nc.dram_tensor("wvT", [C, C], MDT, kind="ExternalInput").ap()
    bqt_d = nc.dram_tensor("bqt", [P, CK], F32, kind="ExternalInput").ap()
    bkt_d = nc.dram_tensor("bkt", [P, CK], F32, kind="ExternalInput").ap()
    bvb_d = nc.dram_tensor("bvb", [P, C], F32, kind="ExternalInput").ap()
    o_d = nc.dram_tensor("o", [C, N], F32, kind="ExternalOutput").ap()

    with ExitStack() as ctx:
        tc = ctx.enter_context(tile.TileContext(nc))
        consts = ctx.enter_context(tc.tile_pool(name="consts", bufs=1))
        wpool = ctx.enter_context(tc.tile_pool(name="wpool", bufs=1))
        kvraw = ctx.enter_context(tc.tile_pool(name="kvraw", bufs=3))
        persist = ctx.enter_context(tc.tile_pool(name="persist", bufs=1))
        qpool = ctx.enter_context(tc.tile_pool(name="qpool", bufs=2))
        qppool = ctx.enter_context(tc.tile_pool(name="qppool", bufs=2))
        spool = ctx.enter_context(tc.tile_pool(name="spool", bufs=3))
        s16 = ctx.enter_context(tc.tile_pool(name="s16", bufs=MCH))
        opool = ctx.enter_context(tc.tile_pool(name="opool", bufs=4))
        ps_a = ctx.enter_context(tc.tile_pool(name="ps_a", bufs=2, space="PSUM"))
        ps_s = ctx.enter_context(tc.tile_pool(name="ps_s", bufs=2, space="PSUM"))
        ps_r = ctx.enter_context(tc.tile_pool(name="ps_r", bufs=4, space="PSUM"))

        for rep in range(reps):
            # ---- phase A: kp[c, m] = wk @ k + bk, kept in SBUF ----
            # DMA emission order puts the phase-A critical path first so the
            # PE can start ~2us in instead of waiting on all constants.
            # interleave wk chunk i with the first k quarter so the first
            # accumulation group's operands arrive in issue order; k/v are
            # staged in [P, NB] quarters to keep the DMA->PE latency short
            wk_sb, kq0 = [], []
            for i in range(CK):
                t = wpool.tile([P, C], MDT, tag=f"wk{i}", name=f"wk{i}")
                nc.sync.dma_start(t[:], wkT_d[i * P:(i + 1) * P, :])
                wk_sb.append(t)
                t = kvraw.tile([P, NB], MDT, tag=f"kv{i}", name=f"kv{i}")
                nc.scalar.dma_start(t[:], k_d[i * P:(i + 1) * P, 0:NB])
                kq0.append(t)
            bkt = consts.tile([P, CK], F32, tag="bkt", name="bkt")
            nc.sync.dma_start(bkt[:], bkt_d[:])

            kp_sb = [persist.tile([P, N], MDT, tag=f"kp{c}", name=f"kp{c}")
                     for c in range(CK)]
            for hq in range(NBK):
                if hq == 0:
                    kq = kq0
                else:
                    kq = []
                    for i in range(CK):
                        t = kvraw.tile([P, NB], MDT, tag=f"kv{i}",
                                       name=f"kv{i}")
                        nc.sync.dma_start(
                            t[:], k_d[i * P:(i + 1) * P,
                                      hq * NB:(hq + 1) * NB])
                        kq.append(t)
                if hq == 1:
                    # phase-B criticals queue right behind the second k
                    # quarter: wv + first v quarter so phase B can start the
                    # moment phase A drains (wq/bqt wait until phase B)
                    wv_sb, vq0 = [], []
                    for i in range(CK):
                        t = wpool.tile([P, C], MDT, tag=f"wv{i}", name=f"wv{i}")
                        nc.sync.dma_start(t[:], wvT_d[i * P:(i + 1) * P, :])
                        wv_sb.append(t)
                    bvb = consts.tile([P, C], F32, tag="bvb", name="bvb")
                    nc.sync.dma_start(bvb[:], bvb_d[:])
                    for i in range(CK):
                        t = kvraw.tile([P, NB], MDT, tag=f"kv{i}",
                                       name=f"kv{i}")
                        nc.sync.dma_start(t[:], v_d[i * P:(i + 1) * P, 0:NB])
                        vq0.append(t)
                for c in range(CK):
                    ps = ps_a.tile([P, NB], F32, tag="ps_a", name="ps_a")
                    for i in range(CK):
                        nc.tensor.matmul(
                            ps[:],
                            wk_sb[i][:, c * P:(c + 1) * P],
                            kq[i][:],
                            start=(i == 0), stop=(i == CK - 1))
                    nc.scalar.activation(
                        kp_sb[c][:, hq * NB:(hq + 1) * NB],
                        ps[:], ACT_IDENT, bias=bkt[:, c:c + 1])

            # ---- phase B: vpt[m, c] = (wv @ v + bv)^T, kept in SBUF ----
            vpt_sb = [persist.tile([P, C], MDT, tag=f"vpt{m}", name=f"vpt{m}")
                      for m in range(MCH)]
            for hq in range(NBK):
                if hq == 0:
                    vq = vq0
                else:
                    vq = []
                    for i in range(CK):
                        t = kvraw.tile([P, NB], MDT, tag=f"kv{i}",
                                       name=f"kv{i}")
                        nc.sync.dma_start(t[:], v_d[i * P:(i + 1) * P,
                                                    hq * NB:(hq + 1) * NB])
                        vq.append(t)
                if hq == 1:
                    # phase-C weights: needed ~30us later, keep out of the
                    # phase-A/B DMA critical path
                    wq_sb = []
                    for i in range(CK):
                        t = wpool.tile([P, C], MDT, tag=f"wq{i}", name=f"wq{i}")
                        nc.sync.dma_start(t[:], wqT_d[i * P:(i + 1) * P, :])
                        wq_sb.append(t)
                    bqt = consts.tile([P, CK], F32, tag="bqt", name="bqt")
                    nc.sync.dma_start(bqt[:], bqt_d[:])
                if hq == 2:
                    # prefetch q block 0 so phase C starts without a DMA wait
                    qt_cur = []
                    for i in range(CK):
                        t = qpool.tile([P, CBLOCKS[0][1]], QDT, tag=f"qt{i}",
                                       name=f"qt{i}")
                        nc.sync.dma_start(
                            t[:], q_d[i * P:(i + 1) * P, 0:CBLOCKS[0][1]])
                        qt_cur.append(t)
                for ml in range(NB // P):
                    m = hq * (NB // P) + ml
                    ps = ps_s.tile([P, C], F32, tag="ps_s", name="ps_s")
                    for i in range(CK):
                        nc.tensor.matmul(
                            ps[:],
                            vq[i][:, ml * P:(ml + 1) * P],
                            wv_sb[i][:],
                            start=(i == 0), stop=(i == CK - 1))
                    nc.vector.tensor_add(vpt_sb[m][:], ps[:], bvb[:])

            def emit_qp(qt_tiles, w):
                qp_sb = []
                for c in range(CK):
                    ps = ps_a.tile([P, w], F32, tag="ps_a", name="ps_a")
                    for i in range(CK):
                        nc.tensor.matmul(
                            ps[:],
                            wq_sb[i][:, c * P:(c + 1) * P],
                            qt_tiles[i][:],
                            start=(i == 0), stop=(i == CK - 1))
                    qp = qppool.tile([P, w], MDT, tag=f"qp{c}", name=f"qp{c}")
                    nc.scalar.activation(qp[:], ps[:], ACT_IDENT,
                                         bias=bqt[:, c:c + 1])
                    qp_sb.append(qp)
                return qp_sb

            # block 0's qp is computed at the tail of phase B so phase C
            # starts directly with S matmuls
            qp_cur = emit_qp(qt_cur, CBLOCKS[0][1])

            # ---- phase C: per n-block: qp, S, out ----
            # variable block widths: the last block is narrow so its
            # post-matmul tail (residual adds + output DMA) is short
            for bi, (b0, w) in enumerate(CBLOCKS):
                qt = qt_cur
                qp_sb = qp_cur
                if bi + 1 < len(CBLOCKS):
                    n0, nw = CBLOCKS[bi + 1]
                    qt_cur = []
                    for i in range(CK):
                        t = qpool.tile([P, nw], QDT, tag=f"qt{i}",
                                       name=f"qt{i}")
                        nc.sync.dma_start(
                            t[:], q_d[i * P:(i + 1) * P, n0:n0 + nw])
                        qt_cur.append(t)

                r_ps = [ps_r.tile([P, w], F32, tag="ps_r", name="ps_r")
                        for _ in range(CK)]

                def emit_s(m):
                    ps = ps_s.tile([P, w], F32, tag="ps_s", name="ps_s")
                    for c in range(CK):
                        nc.tensor.matmul(
                            ps[:],
                            kp_sb[c][:, m * P:(m + 1) * P],
                            qp_sb[c][:],
                            start=(c == 0), stop=(c == CK - 1))
                    return ps

                def emit_out(c):
                    o_sb = opool.tile([P, w], F32, tag="o", name="o")
                    qres = qt[c][:].bitcast(F32) if mode == "f32r" \
                        else qt[c][:]
                    nc.vector.tensor_add(o_sb[:], r_ps[c][:], qres)
                    eng = nc.sync if c % 2 == 0 else nc.scalar
                    eng.dma_start(o_d[c * P:(c + 1) * P, b0:b0 + w],
                                  o_sb[:])

                last = bi + 1 == len(CBLOCKS)
                # for the last block, accumulate c-chunks 0/1 first, then
                # 2/3 from the kept S tiles, so half the residual+store tail
                # overlaps the second pass's matmuls
                cs1 = (0, 1) if last else range(CK)
                s_keep = []
                s_ps_prev = emit_s(0)
                for m in range(MCH):
                    s_ps_next = emit_s(m + 1) if m + 1 < MCH else None
                    if last:
                        s_sb = s16.tile([P, w], MDT, tag="sl", name="sl")
                    else:
                        s_sb = spool.tile([P, w], MDT, tag="s", name="s")
                    # alternate PSUM->SBUF copies between ACT and DVE
                    if m % 2 == 0:
                        nc.scalar.copy(s_sb[:], s_ps_prev[:])
                    else:
                        nc.vector.tensor_copy(s_sb[:], s_ps_prev[:])
                    s_keep.append(s_sb)
                    for c in cs1:
                        nc.tensor.matmul(
                            r_ps[c][:],
                            vpt_sb[m][:, c * P:(c + 1) * P],
                            s_sb[:],
                            start=(m == 0), stop=(m == MCH - 1))
                    if m == 2 and not last:
                        # next block's qp slots into this block's m-loop;
                        # PE covers it with S/res work already queued
                        qp_cur = emit_qp(qt_cur, CBLOCKS[bi + 1][1])
                    s_ps_prev = s_ps_next

                for c in cs1:
                    emit_out(c)
                if last:
                    for m in range(MCH):
                        for c in (2, 3):
                            nc.tensor.matmul(
                                r_ps[c][:],
                                vpt_sb[m][:, c * P:(c + 1) * P],
                                s_keep[m][:],
                                start=(m == 0), stop=(m == MCH - 1))
                    for c in (2, 3):
                        emit_out(c)

    nc.finalize()
    return nc


_CACHE = {}


MODE = "fp16"


def _get_nc():
    if "nc" not in _CACHE:
        _CACHE["nc"] = build_nc(mode=MODE)
    return _CACHE["nc"]


def _in_maps(q, k, v, wq, bq, wk, bk, wv, bv, mode=None):
    if mode is None:
        mode = MODE
    f32 = lambda x: np.ascontiguousarray(np.asarray(x), dtype=np.float32)
    if mode == "f32r":
        mdt = f32
    else:
        import ml_dtypes
        npdt = ml_dtypes.bfloat16 if mode == "bf16" else np.float16
        mdt = lambda x: np.ascontiguousarray(np.asarray(x), dtype=npdt)
    q = mdt(q)
    k, v = mdt(k), mdt(v)
    wqT = mdt(np.asarray(wq).T)
    wkT = mdt(np.asarray(wk).T)
    wvT = mdt(np.asarray(wv).T)
    bqt = f32(np.asarray(bq).reshape(CK, P).T)
    bkt = f32(np.asarray(bk).reshape(CK, P).T)
    bvb = f32(np.tile(np.asarray(bv)[None, :], (P, 1)))
    return [
        {"q": q[i], "k": k[i], "v": v[i],
         "wqT": wqT, "wkT": wkT, "wvT": wvT,
         "bqt": bqt, "bkt": bkt, "bvb": bvb}
        for i in range(N_CORES)
    ]


def run(inputs, **spmd_kwargs):
    """Run on hardware; returns (output [B,C,N], BassKernelResults)."""
    nc = _get_nc()
    maps = _in_maps(**inputs)
    res = run_bass_kernel_spmd(nc, maps, list(range(N_CORES)), **spmd_kwargs)
    out = np.stack([res.results[i]["o"] for i in range(N_CORES)], axis=0)
    return out, res


def kernel(q, k, v, wq, bq, wk, bk, wv, bv):
    out, _ = run(dict(q=q, k=k, v=v, wq=wq, bq=bq, wk=wk, bk=bk,
                      wv=wv, bv=bv))
    return out
